# revision 18
# baseline (speedup 1.0000x reference)
"""Single-head causal self-attention (B=8, T=2048, D=512, H=64), data-parallel
over batch across 8 NeuronCores. Self-contained: builds a Bass/Tile kernel and
runs it via run_bass_kernel_spmd.

Per-core layout (batch element b = core id):
  - x [2048, 512] is PE-transposed to xT (d on partitions, f32r)
  - kT/qT [64, 2048] and v [2048, 64] projections in f32r; Wq, bq pre-scaled
    by H^-0.5 on the host; v is augmented with a ones column so the PV matmul
    also accumulates the softmax denominator
  - attention runs in S^T layout per 512-wide i-block: S^T = kT_chunk^T @ qT,
    exp on ACT (PSUM->SBUF, two j-tiles per instruction), multiplicative
    causal masks on the 4 diagonal j-tiles, PV matmul accumulates [65, 512]
  - epilogue: PE transpose of [65, 128] chunks, reciprocal * row, + bv
    (softmax rows sum to 1, so the v bias folds into the output)
"""

import sys

for _p in ("/root/.axon_site/_ro/trn_rl_repo", "/opt/trn_rl_repo"):
    if _p not in sys.path:
        sys.path.append(_p)

import numpy as np
import concourse.bass as bass
import concourse.bacc as bacc
import concourse.tile as tile
from concourse import mybir
from concourse.bass_utils import run_bass_kernel_spmd
from concourse.masks import make_identity

F32 = mybir.dt.float32
F32R = mybir.dt.float32r

B, T, D, H = 8, 2048, 512, 64
NT = T // 128   # 16 t-tiles
ND = D // 128   # 4 d-chunks
NIB = T // 512  # 4 i-blocks
EXP = mybir.ActivationFunctionType.Exp


def build_body(nc, tc, ctx, dram, repeat=1):
    x_d, w_d, bkq_d, bv_d, out_d = dram

    persist = ctx.enter_context(tc.tile_pool(name="persist", bufs=1))
    epool = ctx.enter_context(tc.tile_pool(name="epool", bufs=6))
    otspool = ctx.enter_context(tc.tile_pool(name="otspool", bufs=3))
    opool = ctx.enter_context(tc.tile_pool(name="opool", bufs=3))
    rpool = ctx.enter_context(tc.tile_pool(name="rpool", bufs=3))
    pspool = ctx.enter_context(tc.tile_pool(name="ps", bufs=2, space="PSUM"))
    ps2pool = ctx.enter_context(tc.tile_pool(name="ps2", bufs=2, space="PSUM"))
    otppool = ctx.enter_context(tc.tile_pool(name="otp", bufs=2, space="PSUM"))

    # --- constants ---
    ident = persist.tile([128, 128], F32)
    make_identity(nc, ident[:])

    bkq_sb = persist.tile([64, 2], F32)
    bv_row = persist.tile([1, 64], F32)
    bvB = persist.tile([128, 64], F32)
    nc.gpsimd.dma_start(bkq_sb[:], bkq_d[:])
    nc.gpsimd.dma_start(bv_row[:], bv_d[:])
    nc.gpsimd.partition_broadcast(bvB[:], bv_row[:])

    # weights -> f32r (packed [ND, 128, 3*64]: k | q | v along last axis)
    wstage = persist.tile([128, ND, 3 * H], F32)
    nc.gpsimd.dma_start(wstage[:], w_d.rearrange("a p h -> p a h"))
    w_r = persist.tile([128, ND, 3 * H], F32R)
    nc.vector.tensor_copy(w_r[:], wstage[:])

    ones_col = persist.tile([128, 1], F32)
    nc.vector.memset(ones_col[:], 1.0)

    # persistent activations
    x_all = persist.tile([128, NT, D], F32)
    xT = persist.tile([128, ND, T], F32R)     # x transposed, d on partitions
    kT = persist.tile([64, T], F32R)
    qT = persist.tile([64, T], F32R)
    vTs = persist.tile([64, T], F32)
    v_aug = persist.tile([128, NT, 65], F32R)  # v rows + ones column
    o_all = persist.tile([128, NT, 64], F32)

    for rep in range(repeat):
        for jt in range(NT):
            nc.vector.tensor_copy(v_aug[:, jt, 64:65], ones_col[:])

        # x in: staged DMAs, small first so transposes start early
        t0 = 0
        for gi, ntile in enumerate((2, 2, 4, 4, 4)):
            eng = nc.sync if gi % 2 == 0 else nc.scalar
            eng.dma_start(
                x_all[:, t0:t0 + ntile, :],
                x_d[128 * t0:128 * (t0 + ntile), :].rearrange(
                    "(a p) d -> p a d", p=128),
            )
            t0 += ntile

        # per 512-wide t-chunk: transpose x, project k/q/v, build v_aug
        for tch in range(4):
            tsl = slice(tch * 512, (tch + 1) * 512)
            for dc in range(ND):
                tp = pspool.tile([128, 4, 128], F32, tag="ps")
                for q in range(4):
                    ti = 4 * tch + q
                    nc.tensor.transpose(
                        tp[:, q, :], x_all[:, ti, dc * 128:(dc + 1) * 128],
                        ident[:])
                nc.vector.tensor_copy(xT[:, dc, tsl], tp[:])

            k_ps = pspool.tile([64, 512], F32, tag="ps")
            for dc in range(ND):
                nc.tensor.matmul(k_ps[:], w_r[:, dc, 0:64], xT[:, dc, tsl],
                                 start=(dc == 0), stop=(dc == ND - 1))
            nc.vector.tensor_scalar_add(kT[:, tsl], k_ps[:], bkq_sb[:, 0:1])

            q_ps = pspool.tile([64, 512], F32, tag="ps")
            for dc in range(ND):
                nc.tensor.matmul(q_ps[:], w_r[:, dc, 64:128], xT[:, dc, tsl],
                                 start=(dc == 0), stop=(dc == ND - 1))
            nc.vector.tensor_scalar_add(qT[:, tsl], q_ps[:], bkq_sb[:, 1:2])

            v_ps = pspool.tile([64, 512], F32, tag="ps")
            for dc in range(ND):
                nc.tensor.matmul(v_ps[:], w_r[:, dc, 128:192], xT[:, dc, tsl],
                                 start=(dc == 0), stop=(dc == ND - 1))
            nc.vector.tensor_copy(vTs[:, tsl], v_ps[:])

            va_ps = pspool.tile([128, 4, 64], F32, tag="ps")
            for q in range(4):
                jt = 4 * tch + q
                nc.tensor.transpose(va_ps[:, q, :], vTs[:, jt * 128:(jt + 1) * 128],
                                    ident[0:64, 0:64])
            nc.vector.tensor_copy(v_aug[:, 4 * tch:4 * tch + 4, 0:64], va_ps[:])

        # --- attention per 512-wide i-block, S^T layout, j-tiles in pairs ---
        for bi in range(NIB):
            isl = slice(bi * 512, (bi + 1) * 512)
            njt = 4 * (bi + 1)
            ot_ps = otppool.tile([65, 512], F32, tag="ot")
            for jp in range(njt // 2):
                st2 = ps2pool.tile([128, 2, 512], F32, tag="ps2")
                for h in range(2):
                    jt = 2 * jp + h
                    nc.tensor.matmul(st2[:, h, :], kT[:, jt * 128:(jt + 1) * 128],
                                     qT[:, isl], start=True, stop=True)
                e2 = epool.tile([128, 2, 512], F32R, tag="e")
                nc.scalar.activation(e2[:], st2[:], EXP)
                for h in range(2):
                    jt = 2 * jp + h
                    if jt >= 4 * bi:
                        nc.gpsimd.affine_select(
                            out=e2[:, h, :], in_=e2[:, h, :],
                            compare_op=mybir.AluOpType.is_ge, fill=0.0,
                            base=-128 * (jt - 4 * bi),
                            pattern=[[1, 512]], channel_multiplier=-1)
                    nc.tensor.matmul(ot_ps[:], v_aug[:, jt, :], e2[:, h, :],
                                     start=(jt == 0), stop=(jt == njt - 1))

            ots = otspool.tile([65, 512], F32, tag="ots")
            nc.vector.tensor_copy(ots[:], ot_ps[:])
            for c in range(4):
                o_ps = pspool.tile([128, 65], F32, tag="ps")
                nc.tensor.transpose(o_ps[:], ots[:, c * 128:(c + 1) * 128],
                                    ident[0:65, 0:65])
                rec = rpool.tile([128, 1], F32, tag="r")
                nc.vector.reciprocal(rec[:], o_ps[:, 64:65])
                it = bi * 4 + c
                nc.vector.tensor_scalar_mul(o_all[:, it, :], o_ps[:, 0:64], rec[:])
                nc.vector.tensor_add(o_all[:, it, :], o_all[:, it, :], bvB[:])

        # output DMA per i-block so the store drains while later blocks run
        for bi in range(NIB):
            nc.sync.dma_start(
                out_d[512 * bi:512 * (bi + 1), :].rearrange(
                    "(a p) h -> p a h", p=128),
                o_all[:, 4 * bi:4 * bi + 4, :])


def build_nc(repeat=1):
    nc = bacc.Bacc("TRN2", target_bir_lowering=False, debug=False, num_devices=8)
    x_d = nc.dram_tensor("x", [T, D], F32, kind="ExternalInput")
    w_d = nc.dram_tensor("w", [ND, 128, 3 * H], F32, kind="ExternalInput")
    bkq_d = nc.dram_tensor("bkq", [H, 2], F32, kind="ExternalInput")
    bv_d = nc.dram_tensor("bv", [1, H], F32, kind="ExternalInput")
    out_d = nc.dram_tensor("out", [T, H], F32, kind="ExternalOutput")
    dram = (x_d, w_d, bkq_d, bv_d, out_d)

    from contextlib import ExitStack
    with tile.TileContext(nc) as tc:
        with ExitStack() as ctx:
            build_body(nc, tc, ctx, dram, repeat=repeat)
    nc.compile()
    return nc


_NC_CACHE = {}


def _get_nc(repeat=1):
    if repeat not in _NC_CACHE:
        _NC_CACHE[repeat] = build_nc(repeat)
    return _NC_CACHE[repeat]


def make_in_maps(x, Wk, bk, Wq, bq, Wv, bv):
    scale = float(H) ** -0.5
    w = np.concatenate(
        [Wk.reshape(ND, 128, H), (Wq * scale).reshape(ND, 128, H),
         Wv.reshape(ND, 128, H)], axis=2)
    w = np.ascontiguousarray(w)
    bkq = np.ascontiguousarray(np.stack([bk, bq * scale], axis=1))
    bvr = np.ascontiguousarray(bv.reshape(1, H))
    return [
        {"x": np.ascontiguousarray(x[b]), "w": w, "bkq": bkq, "bv": bvr}
        for b in range(B)
    ]


def kernel(x, Wk, bk, Wq, bq, Wv, bv, _repeat=1):
    x = np.asarray(x, dtype=np.float32)
    Wk = np.asarray(Wk, dtype=np.float32)
    bk = np.asarray(bk, dtype=np.float32)
    Wq = np.asarray(Wq, dtype=np.float32)
    bq = np.asarray(bq, dtype=np.float32)
    Wv = np.asarray(Wv, dtype=np.float32)
    bv = np.asarray(bv, dtype=np.float32)

    nc = _get_nc(_repeat)
    in_maps = make_in_maps(x, Wk, bk, Wq, bq, Wv, bv)
    res = run_bass_kernel_spmd(nc, in_maps, core_ids=list(range(B)))
    out = np.stack([res.results[b]["out"] for b in range(B)], axis=0)
    return out.astype(np.float32)
